# revision 9
# baseline (speedup 1.0000x reference)
"""GCN (2x shared GCNConv+BN+LeakyReLU, linear head) on 8 trn2 NeuronCores.

v2: nodes row-sharded 8 ways (12500/core, 98 tiles of 128).

Layer 1 aggregates the replicated input x directly (agg commutes with the
linear map): each core holds a host-built fp8 row table [own rows | halo
rows] of x, so there is no z1 table, no layer-1 collective, and no halo z
compute. Aggregation is transposed one-hot matmuls (stationary = gathered
rows slot-major, moving = host-baked one-hot with dinv[src]*dinv[dst]);
self-loop rows ride a separate bf16 copy for precision (fp8 only on the
deg^-1/2-attenuated edge messages). conv1 z = agg_x @ (W1@Wc) is fused
per 4-tile group right after its aggregation, feature-major, so BN stats
fall out per PSUM slab.

Layer 2: BN+LeakyReLU+conv per group writes z2 rows to a bf16 own-row
table (self reads) and a cast fp8 full table (edge gathers + halo
export). Halo rows whose own-row position passes a segment boundary are
gathered+staged immediately and shipped by a per-segment AllToAll that
overlaps the remaining conv groups; the receiving table region is
seg-major so each collective is contiguous. agg2 then runs unchunked.
BN stats: bn_stats per (chunk, group) slab, bn_aggr per layer, 4KB
AllReduce of (sum, sumsq). Head uses folded W2@WO with the 1-column
operand stationary (N=512 moving), sigmoid on the [1, nodes] row.
"""

import math
import os

if os.environ.get("AXON_LOOPBACK_RELAY") or os.environ.get("AXON_POOL_SVC_OVERRIDE"):
    _jp = os.environ.get("JAX_PLATFORMS")
    if _jp and "axon" not in _jp:
        os.environ["JAX_PLATFORMS"] = "axon," + _jp

import numpy as np
import ml_dtypes

from concourse import bacc, bass, mybir, tile
from concourse.bass_utils import run_bass_kernel_spmd

BF16 = mybir.dt.bfloat16
FP8 = mybir.dt.float8e4
F32 = mybir.dt.float32
I16 = mybir.dt.int16
NP_BF16 = ml_dtypes.bfloat16
NP_FP8 = ml_dtypes.float8_e4m3

P = 128
EPS = 1e-5
ALPHA = 0.01
GRP = 4


def _wrap_idx(flat):
    n = len(flat)
    assert n % 16 == 0
    w = np.zeros((16, n // 16), np.int16)
    w[np.arange(n) % 16, np.arange(n) // 16] = flat.astype(np.int16)
    return np.ascontiguousarray(np.tile(w, (8, 1)))


def _ceil16(n):
    return (n + 15) // 16 * 16


# ---------------------------------------------------------------------------
# Host-side planning
# ---------------------------------------------------------------------------

def make_plan(x, edge_index, W1, b1, Wc, bc, gamma, beta, W2, b2, WO, bO, C=8):
    x = np.asarray(x, np.float32)
    ei = np.asarray(edge_index).astype(np.int64)
    src, dst = ei[0], ei[1]
    N, F = x.shape
    H = np.asarray(Wc).shape[0]
    CH = H // P
    S = N // C
    T_OWN = math.ceil(S / P)
    OWN_PAD = T_OWN * P
    NG = math.ceil(T_OWN / GRP)

    deg = np.bincount(dst, minlength=N).astype(np.float64) + 1.0
    dinv = 1.0 / np.sqrt(deg)

    owner_s = src // S
    owner_d = dst // S

    # cross-shard unique sources per (src core j -> dst core k)
    U = [[None] * C for _ in range(C)]
    for k in range(C):
        m = owner_d == k
        es = src[m]
        for j in range(C):
            if j == k:
                continue
            U[j][k] = np.unique(es[owner_s[m] == j])

    # fill segment configs (in z2 producer groups); first that fits int16 wins
    def cfg_frac(fr):
        out = []
        for f in fr:
            g = max(1, min(NG, math.ceil(NG * f)))
            if not out or g > out[-1]:
                out.append(g)
        if out[-1] != NG:
            out.append(NG)
        return tuple(out)

    SEG_CFGS = [cfg_frac((0.48, 0.80, 0.96, 1.0)),
                cfg_frac((0.48, 0.88, 1.0)),
                cfg_frac((0.64, 1.0)),
                (NG,)]

    for cfg in SEG_CFGS:
        bounds = [min(g * GRP * P, OWN_PAD) for g in cfg]
        NSEG = len(bounds)
        SEGS = np.zeros(NSEG, np.int64)
        for j in range(C):
            for k in range(C):
                if j == k:
                    continue
                loc = U[j][k] - j * S
                prev = 0
                for s, b in enumerate(bounds):
                    hi = int(np.searchsorted(loc, b))
                    SEGS[s] = max(SEGS[s], hi - prev)
                    prev = hi
        SEGS = [int(_ceil16(v)) for v in SEGS]
        R = sum(SEGS)
        TABLE_ROWS = OWN_PAD + C * R
        if TABLE_ROWS <= 32767:
            break
    assert TABLE_ROWS <= 32767, TABLE_ROWS
    OFF = np.concatenate([[0], np.cumsum(SEGS)])
    # seg-major halo region: seg s block at OWN_PAD + C*OFF[s], slot j inside
    SEG_BASE = [OWN_PAD + C * int(OFF[s]) for s in range(NSEG)]

    def table_pos(j, k, u):
        """table position of halo src u (sorted array) for pair (j -> k)."""
        ujk = U[j][k]
        loc_all = ujk - j * S
        r = np.searchsorted(ujk, u)
        loc = u - j * S
        pos = np.empty(len(u), np.int64)
        prev = 0
        for s, b in enumerate(bounds):
            hi = int(np.searchsorted(loc_all, b))
            msk = (loc >= (bounds[s - 1] if s else 0)) & (loc < b)
            pos[msk] = SEG_BASE[s] + j * SEGS[s] + (r[msk] - prev)
            prev = hi
        return pos

    # ---- folded weights
    W1 = np.asarray(W1, np.float64)
    Wc64 = np.asarray(Wc, np.float64)
    Wf = (W1 @ Wc64).astype(np.float32)
    bf_row = (np.asarray(b1, np.float64) @ Wc64).astype(np.float32)
    WfO = (np.asarray(W2, np.float64) @ np.asarray(WO, np.float64)).astype(np.float32)
    bOf = float(np.asarray(b2, np.float64) @ np.asarray(WO, np.float64)[:, 0]
                + np.asarray(bO, np.float64)[0])
    has_bias = bool(np.any(bf_row != 0.0))

    def _wlayout(W):
        return np.ascontiguousarray(
            W.reshape(CH, P, H).transpose(1, 0, 2).reshape(P, CH * H).astype(NP_BF16))

    wf_host = _wlayout(Wf)
    wc_host = _wlayout(np.asarray(Wc, np.float32))
    wfo_host = np.ascontiguousarray(WfO.reshape(CH, P).T.astype(NP_BF16))
    gmb_host = np.concatenate(
        [np.asarray(gamma, np.float32).reshape(CH, P).T,
         np.asarray(beta, np.float32).reshape(CH, P).T], axis=1)
    brow_host = bf_row.reshape(1, H).astype(NP_BF16)

    # ---- per-tile edge block structure (shared across cores)
    per_core = []
    blocks_max = np.ones(T_OWN, np.int64)
    for k in range(C):
        m = owner_d == k
        es, ed = src[m], dst[m] - k * S
        order = np.argsort(ed, kind="stable")
        es, ed = es[order], ed[order]
        cnt = np.bincount(ed // P, minlength=T_OWN)
        blocks_max = np.maximum(blocks_max, np.ceil(cnt / P).astype(np.int64))
        per_core.append((es, ed))
    blk_off = np.zeros(T_OWN + 1, np.int64)
    np.cumsum(blocks_max, out=blk_off[1:])
    TOT_BLK = int(blk_off[-1])

    # merged one-hot layout: per group [self gn*P | edge nb*P]
    oh_goff = []
    off = 0
    for g in range(NG):
        g0 = g * GRP
        gn = min(GRP, T_OWN - g0)
        nb = int(blk_off[min(g0 + gn, T_OWN)] - blk_off[g0])
        oh_goff.append(off)
        off += (gn + nb) * P
    OH_COLS = off

    in_maps = []
    for k in range(C):
        es, ed = per_core[k]
        row = np.zeros(len(es), np.int64)
        mloc = (es // S) == k
        row[mloc] = es[mloc] - k * S
        for j in range(C):
            if j == k:
                continue
            mj = (es // S) == j
            if mj.any():
                row[mj] = table_pos(j, k, es[mj])

        tile_id = ed // P
        starts = np.searchsorted(tile_id, np.arange(T_OWN))
        r_in_tile = np.arange(len(ed)) - starts[tile_id]
        gflat = np.zeros(TOT_BLK * P, np.int64)
        gflat[blk_off[tile_id] * P + r_in_tile] = row
        gidx = _wrap_idx(gflat)

        oh_all = np.zeros((P, OH_COLS), NP_BF16)
        dv2 = (dinv[k * S:(k + 1) * S] ** 2).astype(np.float32)
        val = (dinv[es] * dinv[ed + k * S]).astype(np.float32)
        for g in range(NG):
            g0 = g * GRP
            gn = min(GRP, T_OWN - g0)
            base = oh_goff[g]
            for q in range(gn):
                t = g0 + q
                nreal = min(S - t * P, P)
                if nreal > 0:
                    idx = np.arange(nreal)
                    oh_all[idx, base + q * P + idx] = dv2[t * P:t * P + nreal]
            ebase = base + gn * P
            msk = (tile_id >= g0) & (tile_id < g0 + gn)
            bloc = (blk_off[tile_id[msk]] - blk_off[g0] + r_in_tile[msk] // P)
            oh_all[r_in_tile[msk] % P,
                   ebase + bloc * P + (ed[msk] % P)] = val[msk]

        # fill idx array: seg-major, then slot j: own-row positions of
        # U[k][j] within seg s, zero-padded to SEGS[s]
        cols = []
        for s in range(NSEG):
            lo = bounds[s - 1] if s else 0
            for j in range(C):
                seg = np.zeros(SEGS[s], np.int64)
                if j != k:
                    loc = U[k][j] - k * S
                    sel = loc[(loc >= lo) & (loc < bounds[s])]
                    seg[:len(sel)] = sel
                cols.append(seg)
        agidx = _wrap_idx(np.concatenate(cols))

        # layer-1 edge rows pre-gathered on host into slot order (fp8),
        # own rows bf16 row-major for the self matmuls
        gsrc = np.zeros(TOT_BLK * P, np.int64)
        gsrc[blk_off[tile_id] * P + r_in_tile] = es
        xg = np.ascontiguousarray(
            x[gsrc].reshape(TOT_BLK, P, F).transpose(1, 0, 2)
            .reshape(P, TOT_BLK * F).astype(NP_FP8))
        x_own = np.zeros((OWN_PAD, F), np.float32)
        x_own[:S] = x[k * S:(k + 1) * S]
        x_own = np.ascontiguousarray(x_own.astype(NP_BF16))

        im = {
            "xg": xg, "x_own": x_own, "gidx": gidx, "oh_all": oh_all,
            "agidx": agidx, "wf": wf_host, "wc": wc_host, "wfo": wfo_host,
            "gmb": gmb_host,
        }
        if has_bias:
            rowsum = np.zeros(OWN_PAD, np.float64)
            m = owner_d == k
            np.add.at(rowsum, dst[m] - k * S, dinv[src[m]] * dinv[dst[m]])
            rowsum[:S] += dinv[k * S:(k + 1) * S] ** 2
            im["brow"] = brow_host
            im["rowsum"] = np.ascontiguousarray(
                rowsum.reshape(1, OWN_PAD).astype(NP_BF16))
        in_maps.append(im)

    dims = dict(N=N, H=H, CH=CH, C=C, S=S, T_OWN=T_OWN, OWN_PAD=OWN_PAD,
                NG=NG, SEGS=tuple(SEGS), SEG_CFG=tuple(cfg),
                SEG_BASE=tuple(SEG_BASE), BOUNDS=tuple(bounds),
                TABLE_ROWS=TABLE_ROWS, TOT_BLK=TOT_BLK,
                blk_off=tuple(int(v) for v in blk_off),
                oh_goff=tuple(oh_goff), OH_COLS=OH_COLS,
                bOf=bOf, has_bias=has_bias)
    return dims, in_maps


# ---------------------------------------------------------------------------
# Device program
# ---------------------------------------------------------------------------

def build_program(d, n_queues=4):
    C, H, CH = d["C"], d["H"], d["CH"]
    S, T_OWN, OWN_PAD, NG = d["S"], d["T_OWN"], d["OWN_PAD"], d["NG"]
    SEGS, SEG_CFG = d["SEGS"], d["SEG_CFG"]
    SEG_BASE, BOUNDS = d["SEG_BASE"], d["BOUNDS"]
    TABLE_ROWS, TOT_BLK = d["TABLE_ROWS"], d["TOT_BLK"]
    blk_off = d["blk_off"]
    oh_goff = d["oh_goff"]
    OH_COLS = d["OH_COLS"]
    N = d["N"]
    NSEG = len(SEGS)
    OFF = [0]
    for v in SEGS:
        OFF.append(OFF[-1] + v)
    R = OFF[-1]
    groups = [list(range(C))]
    Lrelu = mybir.ActivationFunctionType.Lrelu
    Sqrt = mybir.ActivationFunctionType.Sqrt
    Sigmoid = mybir.ActivationFunctionType.Sigmoid
    Copy = mybir.ActivationFunctionType.Copy
    Add = mybir.AluOpType.add

    nc = bacc.Bacc("TRN2", target_bir_lowering=False, debug=False,
                   enable_asserts=False, num_devices=C, num_swdge_queues=4)

    xg_d = nc.dram_tensor("xg", [P, TOT_BLK * H], FP8, kind="ExternalInput")
    x_own_d = nc.dram_tensor("x_own", [OWN_PAD, H], BF16, kind="ExternalInput")
    gidx_d = nc.dram_tensor("gidx", [P, TOT_BLK * P // 16], I16, kind="ExternalInput")
    agidx_d = nc.dram_tensor("agidx", [P, C * R // 16], I16, kind="ExternalInput")
    oh_d = nc.dram_tensor("oh_all", [P, OH_COLS], BF16, kind="ExternalInput")
    wf_d = nc.dram_tensor("wf", [P, CH * H], BF16, kind="ExternalInput")
    wc_d = nc.dram_tensor("wc", [P, CH * H], BF16, kind="ExternalInput")
    wfo_d = nc.dram_tensor("wfo", [P, CH], BF16, kind="ExternalInput")
    gmb_d = nc.dram_tensor("gmb", [P, 2 * CH], F32, kind="ExternalInput")
    if d["has_bias"]:
        brow_d = nc.dram_tensor("brow", [1, H], BF16, kind="ExternalInput")
        rowsum_d = nc.dram_tensor("rowsum", [1, OWN_PAD], BF16,
                                  kind="ExternalInput")
    out_ext = nc.dram_tensor("out", [S, 1], F32, kind="ExternalOutput")

    def cdiv(a, b):
        return (a + b - 1) // b

    with tile.TileContext(nc) as tc:
        with (
            tc.tile_pool(name="consts", bufs=1) as cp,
            tc.tile_pool(name="work", bufs=2) as wp,
            tc.tile_pool(name="psum", bufs=1, space="PSUM") as pp,
            tc.tile_pool(name="dram", bufs=1, space="DRAM") as dp,
        ):
            # ---- constants
            gidx_sb = cp.tile([P, TOT_BLK * P // 16], I16, name="gidx_sb")
            nc.sync.dma_start(out=gidx_sb, in_=gidx_d[:, :])
            agidx_sb = cp.tile([P, C * R // 16], I16, name="agidx_sb")
            nc.sync.dma_start(out=agidx_sb, in_=agidx_d[:, :])
            wf_sb = cp.tile([P, CH * H], BF16, name="wf_sb")
            nc.sync.dma_start(out=wf_sb, in_=wf_d[:, :])
            wc_sb = cp.tile([P, CH * H], BF16, name="wc_sb")
            nc.sync.dma_start(out=wc_sb, in_=wc_d[:, :])
            wfo_sb = cp.tile([P, CH], BF16, name="wfo_sb")
            nc.sync.dma_start(out=wfo_sb, in_=wfo_d[:, :])
            gmb_sb = cp.tile([P, 2 * CH], F32, name="gmb_sb")
            nc.sync.dma_start(out=gmb_sb, in_=gmb_d[:, :])
            if d["has_bias"]:
                brow_sb = cp.tile([1, H], BF16, name="brow_sb")
                nc.sync.dma_start(out=brow_sb, in_=brow_d[:, :])
                rowsum_sb = cp.tile([1, OWN_PAD], BF16, name="rowsum_sb")
                nc.sync.dma_start(out=rowsum_sb, in_=rowsum_d[:, :])

            aggT_sb = cp.tile([P, NG * CH * GRP * P], BF16, name="aggT_sb")
            stats_sb = [cp.tile([P, NG * CH * 6], F32, name=f"stats{l}")
                        for l in range(2)]

            # ---- DRAM internals
            table2o = dp.tile([OWN_PAD, H], BF16, name="table2o")
            table2 = dp.tile([TABLE_ROWS, H], FP8, name="table2")
            a2a_in = [dp.tile([C * SEGS[s], H], FP8, name=f"a2a_in{s}")
                      for s in range(NSEG)]
            ar_in = [dp.tile([P, 2 * CH], F32, name=f"ar_in{l}")
                     for l in range(2)]
            ar_out = [dp.tile([P, 2 * CH], F32, addr_space="Shared",
                              name=f"ar_out{l}") for l in range(2)]

            qrr = [0]

            def next_q():
                q = qrr[0]
                qrr[0] = (qrr[0] + 1) % n_queues
                return q

            # ================= fills + per-seg A2A =================
            def fill_and_a2a(s):
                ni = SEGS[s]
                if ni == 0:
                    return
                bound = BOUNDS[s]
                pieces = []
                for sl in range(C):
                    for c0 in range(0, ni, 512):
                        pieces.append((sl, c0, min(512, ni - c0)))
                ggs = {}

                def gather(pi):
                    sl, c0, cn = pieces[pi]
                    gg = wp.tile([P, cdiv(cn, P) * H], FP8, tag="gg",
                                 bufs=3, name=f"gg_{s}_{pi}")
                    i0 = C * OFF[s] + sl * SEGS[s] + c0
                    nc.gpsimd.dma_gather(
                        out_ap=gg.rearrange("p (b h) -> p b h", h=H),
                        in_ap=table2[0:bound, :],
                        idxs_ap=agidx_sb[:, i0 // 16:(i0 + cn) // 16],
                        num_idxs=cn, num_idxs_reg=cn, elem_size=H,
                        queue_num=next_q())
                    ggs[pi] = gg

                def stage(pi):
                    sl, c0, cn = pieces[pi]
                    gg = ggs.pop(pi)
                    base = sl * SEGS[s] + c0
                    full = cn // P
                    rem = cn - full * P
                    if full:
                        nc.scalar.dma_start(
                            out=a2a_in[s][base:base + full * P, :].rearrange(
                                "(b p) h -> p b h", p=P),
                            in_=gg.rearrange("p (b h) -> p b h",
                                             h=H)[:, 0:full, :])
                    if rem:
                        nc.scalar.dma_start(
                            out=a2a_in[s][base + full * P:base + cn, :].rearrange(
                                "(b p) h -> p b h", p=rem),
                            in_=gg.rearrange("p (b h) -> p b h",
                                             h=H)[0:rem, full:full + 1, :])

                npc = len(pieces)
                for pi in range(min(2, npc)):
                    gather(pi)
                for pi in range(npc):
                    if pi + 2 < npc:
                        gather(pi + 2)
                    stage(pi)

                nc.gpsimd.collective_compute(
                    "AllToAll", mybir.AluOpType.bypass, replica_groups=groups,
                    ins=[a2a_in[s].opt()],
                    outs=[table2[SEG_BASE[s]:SEG_BASE[s] + C * SEGS[s], :]])

            # ================= aggregation (shared by both layers) ===========
            def agg_group(g, lab, src_rows, src_self, stats, conv_after):
                """Aggregate group g: one-hot matmuls -> pg (feature-major),
                then hand the CH PSUM slabs to conv_after(g, gn, pgs)."""
                g0 = g * GRP
                gn = min(GRP, T_OWN - g0)
                nb = blk_off[g0 + gn] - blk_off[g0]
                ohg = wp.tile([P, (gn + nb) * P], BF16, tag="ohg", bufs=2,
                              name=f"ohg_{lab}_{g0}")
                nc.sync.dma_start(
                    out=ohg,
                    in_=oh_d[:, oh_goff[g]:oh_goff[g] + (gn + nb) * P])
                selfg = wp.tile([P, gn * H], BF16, tag="selfg", bufs=2,
                                name=f"selfg_{lab}_{g0}")
                nc.sync.dma_start(
                    out=selfg.rearrange("p (q h) -> p q h", h=H),
                    in_=src_self[g0 * P:(g0 + gn) * P, :].rearrange(
                        "(q p) h -> p q h", p=P))
                # edge rows: pre-gathered input (layer 1) or dma_gather
                # from the z2 table (layer 2), pieces of <= 8 blocks
                g4s = []
                for b0 in range(0, nb, 8):
                    bn_ = min(8, nb - b0)
                    g4 = wp.tile([P, bn_ * H], FP8, tag="g4", bufs=3,
                                 name=f"g4_{lab}_{g0}_{b0}")
                    if src_rows is None:
                        nc.sync.dma_start(
                            out=g4,
                            in_=xg_d[:, (blk_off[g0] + b0) * H:
                                     (blk_off[g0] + b0 + bn_) * H])
                    else:
                        ni = bn_ * P
                        i0 = (blk_off[g0] + b0) * P
                        nc.gpsimd.dma_gather(
                            out_ap=g4.rearrange("p (b h) -> p b h", h=H),
                            in_ap=src_rows,
                            idxs_ap=gidx_sb[:, i0 // 16:(i0 + ni) // 16],
                            num_idxs=ni, num_idxs_reg=ni, elem_size=H,
                            queue_num=next_q())
                    g4s.append(g4)
                outs = []
                for c in range(CH):
                    pg = pp.tile([P, GRP * P], F32, tag="agg", bufs=4,
                                 name=f"agg_{lab}_{g0}_{c}")
                    for q in range(gn):
                        t = g0 + q
                        nbt = blk_off[t + 1] - blk_off[t]
                        bb = blk_off[t] - blk_off[g0]
                        nc.tensor.matmul(
                            pg[:, q * P:(q + 1) * P],
                            lhsT=selfg[:, q * H + c * P:q * H + (c + 1) * P],
                            rhs=ohg[:, q * P:(q + 1) * P],
                            start=True, stop=(nbt == 0),
                            skip_group_check=True)
                        for b in range(nbt):
                            gb = bb + b
                            g4 = g4s[gb // 8]
                            lb = gb % 8
                            nc.tensor.matmul(
                                pg[:, q * P:(q + 1) * P],
                                lhsT=g4[:, lb * H + c * P:lb * H + (c + 1) * P],
                                rhs=ohg[:, (gn + gb) * P:(gn + gb + 1) * P],
                                start=False, stop=(b == nbt - 1),
                                skip_group_check=True)
                    if stats is not None:
                        nc.vector.bn_stats(
                            stats[:, (c * NG + g) * 6:(c * NG + g) * 6 + 6],
                            pg[:, 0:gn * P])
                    outs.append(pg)
                conv_after(g, gn, outs)

            # layer 1: agg over x then fused conv1 (feature-major)
            def conv1_after(g, gn, pgs):
                g0 = g * GRP
                xagg = wp.tile([P, CH * gn * P], BF16, tag="xagg", bufs=2,
                               name=f"xagg_{g0}")
                for c in range(CH):
                    nc.vector.tensor_copy(
                        xagg[:, c * gn * P:(c + 1) * gn * P],
                        pgs[c][:, 0:gn * P])
                for co in range(CH):
                    zt = pp.tile([P, GRP * P], F32, tag="zp", bufs=3,
                                 name=f"zt_{g0}_{co}")
                    for ci in range(CH):
                        last = (ci == CH - 1) and not d["has_bias"]
                        nc.tensor.matmul(
                            zt[:, 0:gn * P],
                            lhsT=wf_sb[:, ci * H + co * P:ci * H + (co + 1) * P],
                            rhs=xagg[:, ci * gn * P:(ci + 1) * gn * P],
                            start=(ci == 0), stop=last)
                    if d["has_bias"]:
                        nc.tensor.matmul(
                            zt[:, 0:gn * P],
                            lhsT=brow_sb[:, co * P:(co + 1) * P],
                            rhs=rowsum_sb[:, g0 * P:g0 * P + gn * P],
                            start=False, stop=True)
                    nc.vector.bn_stats(
                        stats_sb[0][:, (co * NG + g) * 6:(co * NG + g) * 6 + 6],
                        zt[:, 0:gn * P])
                    nc.scalar.activation(
                        aggT_sb[:, (g * CH + co) * GRP * P:
                                (g * CH + co) * GRP * P + gn * P],
                        zt[:, 0:gn * P], Copy)

            def agg2_after(g, gn, pgs):
                g0 = g * GRP
                for c in range(CH):
                    nc.scalar.activation(
                        aggT_sb[:, (g * CH + c) * GRP * P:
                                (g * CH + c) * GRP * P + gn * P],
                        pgs[c][:, 0:gn * P], Copy)

            # ================= BN coeffs =================
            def bn_coeffs(l):
                stats = stats_sb[l]
                mv = cp.tile([P, 2 * CH], F32, name=f"mv_{l}")
                for c in range(CH):
                    nc.vector.bn_aggr(
                        mv[:, 2 * c:2 * c + 2],
                        stats[:, c * NG * 6:(c + 1) * NG * 6])
                mu_l = mv.rearrange("p (c two) -> p c two", two=2)[:, :, 0]
                var_l = mv.rearrange("p (c two) -> p c two", two=2)[:, :, 1]
                sums = cp.tile([P, 2 * CH], F32, name=f"sums_{l}")
                m2 = cp.tile([P, CH], F32, name=f"m2_{l}")
                nc.vector.tensor_mul(m2, mu_l, mu_l)
                nc.vector.tensor_add(m2, var_l, m2)
                nc.vector.tensor_scalar_mul(sums[:, 0:CH], mu_l, float(OWN_PAD))
                nc.vector.tensor_scalar_mul(sums[:, CH:2 * CH], m2,
                                            float(OWN_PAD))
                nc.sync.dma_start(out=ar_in[l][:, :], in_=sums)
                nc.gpsimd.collective_compute(
                    "AllReduce", Add, replica_groups=groups,
                    ins=[ar_in[l].opt()], outs=[ar_out[l].opt()])
                sg = cp.tile([P, 2 * CH], F32, name=f"sg_{l}")
                nc.sync.dma_start(out=sg, in_=ar_out[l][:, :])
                mu = cp.tile([P, CH], F32, name=f"mu_{l}")
                nc.vector.tensor_scalar_mul(mu, sg[:, 0:CH], 1.0 / N)
                ex2 = cp.tile([P, CH], F32, name=f"ex2_{l}")
                nc.vector.tensor_scalar_mul(ex2, sg[:, CH:2 * CH], 1.0 / N)
                var = cp.tile([P, CH], F32, name=f"var_{l}")
                nc.vector.tensor_mul(var, mu, mu)
                nc.vector.tensor_sub(var, ex2, var)
                nc.vector.tensor_scalar_add(var, var, EPS)
                std = cp.tile([P, CH], F32, name=f"std_{l}")
                nc.scalar.activation(std, var, Sqrt)
                rstd = cp.tile([P, CH], F32, name=f"rstd_{l}")
                nc.vector.reciprocal(rstd, std)
                scale = cp.tile([P, CH], F32, name=f"scale_{l}")
                nc.vector.tensor_mul(scale, gmb_sb[:, 0:CH], rstd)
                shift = cp.tile([P, CH], F32, name=f"shift_{l}")
                nc.vector.tensor_mul(shift, mu, scale)
                nc.vector.tensor_sub(shift, gmb_sb[:, CH:2 * CH], shift)
                return scale, shift

            # ================= z2 producer =================
            def z2_phase(scale, shift):
                seg_ptr = [0]
                for g in range(NG):
                    g0 = g * GRP
                    gn = min(GRP, T_OWN - g0)
                    ht = wp.tile([P, CH * gn * P], BF16, tag="ht", bufs=2,
                                 name=f"ht_{g0}")
                    for c in range(CH):
                        nc.scalar.activation(
                            ht[:, c * gn * P:(c + 1) * gn * P],
                            aggT_sb[:, (g * CH + c) * GRP * P:
                                    (g * CH + c) * GRP * P + gn * P],
                            Lrelu, bias=shift[:, c:c + 1],
                            scale=scale[:, c:c + 1], alpha=ALPHA)
                    zh4 = wp.tile([P, gn * H], BF16, tag="zh4", bufs=2,
                                  name=f"zh4_{g0}")
                    zh8 = wp.tile([P, gn * H], FP8, tag="zh8", bufs=2,
                                  name=f"zh8_{g0}")
                    for q in range(gn):
                        zp = pp.tile([P, H], F32, tag="zp", bufs=3,
                                     name=f"zp2_{g0}_{q}")
                        for c in range(CH):
                            nc.tensor.matmul(
                                zp,
                                lhsT=ht[:, (c * gn + q) * P:
                                        (c * gn + q + 1) * P],
                                rhs=wc_sb[:, c * H:(c + 1) * H],
                                start=(c == 0), stop=(c == CH - 1))
                        nc.vector.tensor_copy(zh4[:, q * H:(q + 1) * H], zp)
                        nc.scalar.activation(zh8[:, q * H:(q + 1) * H], zp,
                                             Copy)
                    nc.sync.dma_start(
                        out=table2o[g0 * P:(g0 + gn) * P, :].rearrange(
                            "(q p) h -> p q h", p=P),
                        in_=zh4.rearrange("p (q h) -> p q h", h=H))
                    nc.sync.dma_start(
                        out=table2[g0 * P:(g0 + gn) * P, :].rearrange(
                            "(q p) h -> p q h", p=P),
                        in_=zh8.rearrange("p (q h) -> p q h", h=H))
                    while (seg_ptr[0] < NSEG
                           and g + 1 == SEG_CFG[seg_ptr[0]]):
                        fill_and_a2a(seg_ptr[0])
                        seg_ptr[0] += 1

            # ================= head =================
            # 4 groups share one PSUM bank (rows 0/32/64/96) so the Lrelu
            # activations batch up and the Sigmoid table swap happens once
            # per batch instead of once per group.
            def head(scale, shift):
                for gb in range(0, NG, 4):
                    gset = list(range(gb, min(gb + 4, NG)))
                    op4 = pp.tile([P, GRP * P], F32, tag="zp", bufs=3,
                                  name=f"op4_{gb}")
                    for gi, g in enumerate(gset):
                        g0 = g * GRP
                        gn = min(GRP, T_OWN - g0)
                        ht = wp.tile([P, CH * gn * P], BF16, tag="ht", bufs=2,
                                     name=f"hto_{g0}")
                        for c in range(CH):
                            nc.scalar.activation(
                                ht[:, c * gn * P:(c + 1) * gn * P],
                                aggT_sb[:, (g * CH + c) * GRP * P:
                                        (g * CH + c) * GRP * P + gn * P],
                                Lrelu, bias=shift[:, c:c + 1],
                                scale=scale[:, c:c + 1], alpha=ALPHA)
                        for c in range(CH):
                            nc.tensor.matmul(
                                op4[32 * gi:32 * gi + 1, 0:gn * P],
                                lhsT=wfo_sb[:, c:c + 1],
                                rhs=ht[:, c * gn * P:(c + 1) * gn * P],
                                start=(c == 0), stop=(c == CH - 1),
                                tile_position=(0, 32 * gi),
                                skip_group_check=True)
                    for gi, g in enumerate(gset):
                        g0 = g * GRP
                        gn = min(GRP, T_OWN - g0)
                        sig = wp.tile([1, GRP * P], F32, tag="sig", bufs=2,
                                      name=f"sig_{g0}")
                        nc.scalar.activation(
                            sig[:, 0:gn * P],
                            op4[32 * gi:32 * gi + 1, 0:gn * P], Sigmoid,
                            bias=float(d["bOf"]), scale=1.0)
                        hi = min(S, g0 * P + gn * P)
                        if hi > g0 * P:
                            nc.sync.dma_start(
                                out=out_ext[g0 * P:hi, :].rearrange(
                                    "(a s) one -> a (s one)", a=1),
                                in_=sig[:, 0:hi - g0 * P])

            # ================= pipeline =================
            for g in range(NG):
                agg_group(g, "l1", None, x_own_d, None,
                          conv1_after)
            scale1, shift1 = bn_coeffs(0)
            z2_phase(scale1, shift1)
            for g in range(NG):
                agg_group(g, "l2", table2[0:TABLE_ROWS, :], table2o,
                          stats_sb[1], agg2_after)
            scale2, shift2 = bn_coeffs(1)
            head(scale2, shift2)

    nc.compile()
    return nc


# ---------------------------------------------------------------------------
# Entry point
# ---------------------------------------------------------------------------

_CACHE = {}


def _get_program(dims):
    key = tuple(sorted((k, str(v)) for k, v in dims.items()))
    if key not in _CACHE:
        _CACHE[key] = build_program(dims)
    return _CACHE[key]


def kernel(x, edge_index, W1, b1, Wc, bc, gamma, beta, W2, b2, WO, bO,
           trace=False):
    dims, in_maps = make_plan(x, edge_index, W1, b1, Wc, bc, gamma, beta,
                              W2, b2, WO, bO)
    nc = _get_program(dims)
    res = run_bass_kernel_spmd(nc, in_maps, core_ids=list(range(dims["C"])),
                               trace=trace)
    out = np.concatenate([r["out"] for r in res.results], axis=0)
    kernel.last_results = res
    return out.astype(np.float32)


# revision 10
# speedup vs baseline: 1.0339x; 1.0339x over previous
"""GCN (2x shared GCNConv+BN+LeakyReLU, linear head) on 8 trn2 NeuronCores.

v2: nodes row-sharded 8 ways (12500/core, 98 tiles of 128).

Layer 1 aggregates the replicated input x directly (agg commutes with the
linear map): each core holds a host-built fp8 row table [own rows | halo
rows] of x, so there is no z1 table, no layer-1 collective, and no halo z
compute. Aggregation is transposed one-hot matmuls (stationary = gathered
rows slot-major, moving = host-baked one-hot with dinv[src]*dinv[dst]);
self-loop rows ride a separate bf16 copy for precision (fp8 only on the
deg^-1/2-attenuated edge messages). conv1 z = agg_x @ (W1@Wc) is fused
per 4-tile group right after its aggregation, feature-major, so BN stats
fall out per PSUM slab.

Layer 2: BN+LeakyReLU+conv per group writes z2 rows to a bf16 own-row
table (self reads) and a cast fp8 full table (edge gathers + halo
export). Halo rows whose own-row position passes a segment boundary are
gathered+staged immediately and shipped by a per-segment AllToAll that
overlaps the remaining conv groups; the receiving table region is
seg-major so each collective is contiguous. agg2 then runs unchunked.
BN stats: bn_stats per (chunk, group) slab, bn_aggr per layer, 4KB
AllReduce of (sum, sumsq). Head uses folded W2@WO with the 1-column
operand stationary (N=512 moving), sigmoid on the [1, nodes] row.
"""

import math
import os

if os.environ.get("AXON_LOOPBACK_RELAY") or os.environ.get("AXON_POOL_SVC_OVERRIDE"):
    _jp = os.environ.get("JAX_PLATFORMS")
    if _jp and "axon" not in _jp:
        os.environ["JAX_PLATFORMS"] = "axon," + _jp

import numpy as np
import ml_dtypes

from concourse import bacc, bass, mybir, tile
from concourse.bass_utils import run_bass_kernel_spmd

BF16 = mybir.dt.bfloat16
FP8 = mybir.dt.float8e4
F32 = mybir.dt.float32
I16 = mybir.dt.int16
NP_BF16 = ml_dtypes.bfloat16
NP_FP8 = ml_dtypes.float8_e4m3

P = 128
EPS = 1e-5
ALPHA = 0.01
GRP = 4


def _wrap_idx(flat):
    n = len(flat)
    assert n % 16 == 0
    w = np.zeros((16, n // 16), np.int16)
    w[np.arange(n) % 16, np.arange(n) // 16] = flat.astype(np.int16)
    return np.ascontiguousarray(np.tile(w, (8, 1)))


def _ceil16(n):
    return (n + 15) // 16 * 16


# ---------------------------------------------------------------------------
# Host-side planning
# ---------------------------------------------------------------------------

def make_plan(x, edge_index, W1, b1, Wc, bc, gamma, beta, W2, b2, WO, bO, C=8):
    x = np.asarray(x, np.float32)
    ei = np.asarray(edge_index).astype(np.int64)
    src, dst = ei[0], ei[1]
    N, F = x.shape
    H = np.asarray(Wc).shape[0]
    CH = H // P
    S = N // C
    T_OWN = math.ceil(S / P)
    OWN_PAD = T_OWN * P
    NG = math.ceil(T_OWN / GRP)

    deg = np.bincount(dst, minlength=N).astype(np.float64) + 1.0
    dinv = 1.0 / np.sqrt(deg)

    owner_s = src // S
    owner_d = dst // S

    # cross-shard unique sources per (src core j -> dst core k)
    U = [[None] * C for _ in range(C)]
    for k in range(C):
        m = owner_d == k
        es = src[m]
        for j in range(C):
            if j == k:
                continue
            U[j][k] = np.unique(es[owner_s[m] == j])

    # fill segment configs (in z2 producer groups); first that fits int16 wins
    def cfg_frac(fr):
        out = []
        for f in fr:
            g = max(1, min(NG, math.ceil(NG * f)))
            if not out or g > out[-1]:
                out.append(g)
        if out[-1] != NG:
            out.append(NG)
        return tuple(out)

    SEG_CFGS = [cfg_frac((0.32, 0.64, 0.88, 1.0)),
                cfg_frac((0.48, 0.80, 0.96, 1.0)),
                cfg_frac((0.48, 0.88, 1.0)),
                cfg_frac((0.64, 1.0)),
                (NG,)]

    for cfg in SEG_CFGS:
        bounds = [min(g * GRP * P, OWN_PAD) for g in cfg]
        NSEG = len(bounds)
        SEGS = np.zeros(NSEG, np.int64)
        for j in range(C):
            for k in range(C):
                if j == k:
                    continue
                loc = U[j][k] - j * S
                prev = 0
                for s, b in enumerate(bounds):
                    hi = int(np.searchsorted(loc, b))
                    SEGS[s] = max(SEGS[s], hi - prev)
                    prev = hi
        SEGS = [int(_ceil16(v)) for v in SEGS]
        R = sum(SEGS)
        TABLE_ROWS = OWN_PAD + C * R
        if TABLE_ROWS <= 32767:
            break
    assert TABLE_ROWS <= 32767, TABLE_ROWS
    OFF = np.concatenate([[0], np.cumsum(SEGS)])
    # seg-major halo region: seg s block at OWN_PAD + C*OFF[s], slot j inside
    SEG_BASE = [OWN_PAD + C * int(OFF[s]) for s in range(NSEG)]

    def table_pos(j, k, u):
        """table position of halo src u (sorted array) for pair (j -> k)."""
        ujk = U[j][k]
        loc_all = ujk - j * S
        r = np.searchsorted(ujk, u)
        loc = u - j * S
        pos = np.empty(len(u), np.int64)
        prev = 0
        for s, b in enumerate(bounds):
            hi = int(np.searchsorted(loc_all, b))
            msk = (loc >= (bounds[s - 1] if s else 0)) & (loc < b)
            pos[msk] = SEG_BASE[s] + j * SEGS[s] + (r[msk] - prev)
            prev = hi
        return pos

    # ---- folded weights
    W1 = np.asarray(W1, np.float64)
    Wc64 = np.asarray(Wc, np.float64)
    Wf = (W1 @ Wc64).astype(np.float32)
    bf_row = (np.asarray(b1, np.float64) @ Wc64).astype(np.float32)
    WfO = (np.asarray(W2, np.float64) @ np.asarray(WO, np.float64)).astype(np.float32)
    bOf = float(np.asarray(b2, np.float64) @ np.asarray(WO, np.float64)[:, 0]
                + np.asarray(bO, np.float64)[0])
    has_bias = bool(np.any(bf_row != 0.0))

    def _wlayout(W):
        return np.ascontiguousarray(
            W.reshape(CH, P, H).transpose(1, 0, 2).reshape(P, CH * H).astype(NP_BF16))

    wf_host = _wlayout(Wf)
    wc_host = _wlayout(np.asarray(Wc, np.float32))
    wfo_host = np.ascontiguousarray(WfO.reshape(CH, P).T.astype(NP_BF16))
    gmb_host = np.concatenate(
        [np.asarray(gamma, np.float32).reshape(CH, P).T,
         np.asarray(beta, np.float32).reshape(CH, P).T], axis=1)
    brow_host = bf_row.reshape(1, H).astype(NP_BF16)

    # ---- per-tile edge block structure (shared across cores)
    per_core = []
    blocks_max = np.ones(T_OWN, np.int64)
    for k in range(C):
        m = owner_d == k
        es, ed = src[m], dst[m] - k * S
        order = np.argsort(ed, kind="stable")
        es, ed = es[order], ed[order]
        cnt = np.bincount(ed // P, minlength=T_OWN)
        blocks_max = np.maximum(blocks_max, np.ceil(cnt / P).astype(np.int64))
        per_core.append((es, ed))
    blk_off = np.zeros(T_OWN + 1, np.int64)
    np.cumsum(blocks_max, out=blk_off[1:])
    TOT_BLK = int(blk_off[-1])

    # merged one-hot layout: per group [self gn*P | edge nb*P]
    oh_goff = []
    off = 0
    for g in range(NG):
        g0 = g * GRP
        gn = min(GRP, T_OWN - g0)
        nb = int(blk_off[min(g0 + gn, T_OWN)] - blk_off[g0])
        oh_goff.append(off)
        off += (gn + nb) * P
    OH_COLS = off

    in_maps = []
    for k in range(C):
        es, ed = per_core[k]
        row = np.zeros(len(es), np.int64)
        mloc = (es // S) == k
        row[mloc] = es[mloc] - k * S
        for j in range(C):
            if j == k:
                continue
            mj = (es // S) == j
            if mj.any():
                row[mj] = table_pos(j, k, es[mj])

        tile_id = ed // P
        starts = np.searchsorted(tile_id, np.arange(T_OWN))
        r_in_tile = np.arange(len(ed)) - starts[tile_id]
        gflat = np.zeros(TOT_BLK * P, np.int64)
        gflat[blk_off[tile_id] * P + r_in_tile] = row
        gidx = _wrap_idx(gflat)

        oh_all = np.zeros((P, OH_COLS), NP_BF16)
        dv2 = (dinv[k * S:(k + 1) * S] ** 2).astype(np.float32)
        val = (dinv[es] * dinv[ed + k * S]).astype(np.float32)
        for g in range(NG):
            g0 = g * GRP
            gn = min(GRP, T_OWN - g0)
            base = oh_goff[g]
            for q in range(gn):
                t = g0 + q
                nreal = min(S - t * P, P)
                if nreal > 0:
                    idx = np.arange(nreal)
                    oh_all[idx, base + q * P + idx] = dv2[t * P:t * P + nreal]
            ebase = base + gn * P
            msk = (tile_id >= g0) & (tile_id < g0 + gn)
            bloc = (blk_off[tile_id[msk]] - blk_off[g0] + r_in_tile[msk] // P)
            oh_all[r_in_tile[msk] % P,
                   ebase + bloc * P + (ed[msk] % P)] = val[msk]

        # fill idx array: seg-major, then slot j: own-row positions of
        # U[k][j] within seg s, zero-padded to SEGS[s]
        cols = []
        for s in range(NSEG):
            lo = bounds[s - 1] if s else 0
            for j in range(C):
                seg = np.zeros(SEGS[s], np.int64)
                if j != k:
                    loc = U[k][j] - k * S
                    sel = loc[(loc >= lo) & (loc < bounds[s])]
                    seg[:len(sel)] = sel
                cols.append(seg)
        agidx = _wrap_idx(np.concatenate(cols))

        # layer-1 edge rows pre-gathered on host into slot order (fp8),
        # own rows bf16 row-major for the self matmuls
        gsrc = np.zeros(TOT_BLK * P, np.int64)
        gsrc[blk_off[tile_id] * P + r_in_tile] = es
        xg = np.ascontiguousarray(
            x[gsrc].reshape(TOT_BLK, P, F).transpose(1, 0, 2)
            .reshape(P, TOT_BLK * F).astype(NP_FP8))
        x_own = np.zeros((OWN_PAD, F), np.float32)
        x_own[:S] = x[k * S:(k + 1) * S]
        x_own = np.ascontiguousarray(x_own.astype(NP_BF16))

        im = {
            "xg": xg, "x_own": x_own, "gidx": gidx, "oh_all": oh_all,
            "agidx": agidx, "wf": wf_host, "wc": wc_host, "wfo": wfo_host,
            "gmb": gmb_host,
        }
        if has_bias:
            rowsum = np.zeros(OWN_PAD, np.float64)
            m = owner_d == k
            np.add.at(rowsum, dst[m] - k * S, dinv[src[m]] * dinv[dst[m]])
            rowsum[:S] += dinv[k * S:(k + 1) * S] ** 2
            im["brow"] = brow_host
            im["rowsum"] = np.ascontiguousarray(
                rowsum.reshape(1, OWN_PAD).astype(NP_BF16))
        in_maps.append(im)

    dims = dict(N=N, H=H, CH=CH, C=C, S=S, T_OWN=T_OWN, OWN_PAD=OWN_PAD,
                NG=NG, SEGS=tuple(SEGS), SEG_CFG=tuple(cfg),
                SEG_BASE=tuple(SEG_BASE), BOUNDS=tuple(bounds),
                TABLE_ROWS=TABLE_ROWS, TOT_BLK=TOT_BLK,
                blk_off=tuple(int(v) for v in blk_off),
                oh_goff=tuple(oh_goff), OH_COLS=OH_COLS,
                bOf=bOf, has_bias=has_bias)
    return dims, in_maps


# ---------------------------------------------------------------------------
# Device program
# ---------------------------------------------------------------------------

def build_program(d, n_queues=4):
    C, H, CH = d["C"], d["H"], d["CH"]
    S, T_OWN, OWN_PAD, NG = d["S"], d["T_OWN"], d["OWN_PAD"], d["NG"]
    SEGS, SEG_CFG = d["SEGS"], d["SEG_CFG"]
    SEG_BASE, BOUNDS = d["SEG_BASE"], d["BOUNDS"]
    TABLE_ROWS, TOT_BLK = d["TABLE_ROWS"], d["TOT_BLK"]
    blk_off = d["blk_off"]
    oh_goff = d["oh_goff"]
    OH_COLS = d["OH_COLS"]
    N = d["N"]
    NSEG = len(SEGS)
    OFF = [0]
    for v in SEGS:
        OFF.append(OFF[-1] + v)
    R = OFF[-1]
    groups = [list(range(C))]
    Lrelu = mybir.ActivationFunctionType.Lrelu
    Sqrt = mybir.ActivationFunctionType.Sqrt
    Sigmoid = mybir.ActivationFunctionType.Sigmoid
    Copy = mybir.ActivationFunctionType.Copy
    Add = mybir.AluOpType.add

    nc = bacc.Bacc("TRN2", target_bir_lowering=False, debug=False,
                   enable_asserts=False, num_devices=C, num_swdge_queues=4)

    xg_d = nc.dram_tensor("xg", [P, TOT_BLK * H], FP8, kind="ExternalInput")
    x_own_d = nc.dram_tensor("x_own", [OWN_PAD, H], BF16, kind="ExternalInput")
    gidx_d = nc.dram_tensor("gidx", [P, TOT_BLK * P // 16], I16, kind="ExternalInput")
    agidx_d = nc.dram_tensor("agidx", [P, C * R // 16], I16, kind="ExternalInput")
    oh_d = nc.dram_tensor("oh_all", [P, OH_COLS], BF16, kind="ExternalInput")
    wf_d = nc.dram_tensor("wf", [P, CH * H], BF16, kind="ExternalInput")
    wc_d = nc.dram_tensor("wc", [P, CH * H], BF16, kind="ExternalInput")
    wfo_d = nc.dram_tensor("wfo", [P, CH], BF16, kind="ExternalInput")
    gmb_d = nc.dram_tensor("gmb", [P, 2 * CH], F32, kind="ExternalInput")
    if d["has_bias"]:
        brow_d = nc.dram_tensor("brow", [1, H], BF16, kind="ExternalInput")
        rowsum_d = nc.dram_tensor("rowsum", [1, OWN_PAD], BF16,
                                  kind="ExternalInput")
    out_ext = nc.dram_tensor("out", [S, 1], F32, kind="ExternalOutput")

    def cdiv(a, b):
        return (a + b - 1) // b

    with tile.TileContext(nc) as tc:
        with (
            tc.tile_pool(name="consts", bufs=1) as cp,
            tc.tile_pool(name="work", bufs=2) as wp,
            tc.tile_pool(name="psum", bufs=1, space="PSUM") as pp,
            tc.tile_pool(name="dram", bufs=1, space="DRAM") as dp,
        ):
            # ---- constants
            gidx_sb = cp.tile([P, TOT_BLK * P // 16], I16, name="gidx_sb")
            nc.sync.dma_start(out=gidx_sb, in_=gidx_d[:, :])
            agidx_sb = cp.tile([P, C * R // 16], I16, name="agidx_sb")
            nc.sync.dma_start(out=agidx_sb, in_=agidx_d[:, :])
            wf_sb = cp.tile([P, CH * H], BF16, name="wf_sb")
            nc.sync.dma_start(out=wf_sb, in_=wf_d[:, :])
            wc_sb = cp.tile([P, CH * H], BF16, name="wc_sb")
            nc.sync.dma_start(out=wc_sb, in_=wc_d[:, :])
            wfo_sb = cp.tile([P, CH], BF16, name="wfo_sb")
            nc.sync.dma_start(out=wfo_sb, in_=wfo_d[:, :])
            gmb_sb = cp.tile([P, 2 * CH], F32, name="gmb_sb")
            nc.sync.dma_start(out=gmb_sb, in_=gmb_d[:, :])
            if d["has_bias"]:
                brow_sb = cp.tile([1, H], BF16, name="brow_sb")
                nc.sync.dma_start(out=brow_sb, in_=brow_d[:, :])
                rowsum_sb = cp.tile([1, OWN_PAD], BF16, name="rowsum_sb")
                nc.sync.dma_start(out=rowsum_sb, in_=rowsum_d[:, :])

            aggT_sb = cp.tile([P, NG * CH * GRP * P], BF16, name="aggT_sb")
            stats_sb = [cp.tile([P, NG * CH * 6], F32, name=f"stats{l}")
                        for l in range(2)]

            # ---- DRAM internals
            table2o = dp.tile([OWN_PAD, H], BF16, name="table2o")
            table2 = dp.tile([TABLE_ROWS, H], FP8, name="table2")
            a2a_in = [dp.tile([C * SEGS[s], H], FP8, name=f"a2a_in{s}")
                      for s in range(NSEG)]
            ar_in = [dp.tile([P, 2 * CH], F32, name=f"ar_in{l}")
                     for l in range(2)]
            ar_out = [dp.tile([P, 2 * CH], F32, addr_space="Shared",
                              name=f"ar_out{l}") for l in range(2)]

            qrr = [0]

            def next_q():
                q = qrr[0]
                qrr[0] = (qrr[0] + 1) % n_queues
                return q

            # ================= fills + per-seg A2A =================
            def fill_and_a2a(s):
                ni = SEGS[s]
                if ni == 0:
                    return
                bound = BOUNDS[s]
                pieces = []
                for sl in range(C):
                    for c0 in range(0, ni, 512):
                        pieces.append((sl, c0, min(512, ni - c0)))
                ggs = {}

                def gather(pi):
                    sl, c0, cn = pieces[pi]
                    gg = wp.tile([P, cdiv(cn, P) * H], FP8, tag="gg",
                                 bufs=3, name=f"gg_{s}_{pi}")
                    i0 = C * OFF[s] + sl * SEGS[s] + c0
                    nc.gpsimd.dma_gather(
                        out_ap=gg.rearrange("p (b h) -> p b h", h=H),
                        in_ap=table2[0:bound, :],
                        idxs_ap=agidx_sb[:, i0 // 16:(i0 + cn) // 16],
                        num_idxs=cn, num_idxs_reg=cn, elem_size=H,
                        queue_num=next_q())
                    ggs[pi] = gg

                def stage(pi):
                    sl, c0, cn = pieces[pi]
                    gg = ggs.pop(pi)
                    base = sl * SEGS[s] + c0
                    full = cn // P
                    rem = cn - full * P
                    if full:
                        nc.scalar.dma_start(
                            out=a2a_in[s][base:base + full * P, :].rearrange(
                                "(b p) h -> p b h", p=P),
                            in_=gg.rearrange("p (b h) -> p b h",
                                             h=H)[:, 0:full, :])
                    if rem:
                        nc.scalar.dma_start(
                            out=a2a_in[s][base + full * P:base + cn, :].rearrange(
                                "(b p) h -> p b h", p=rem),
                            in_=gg.rearrange("p (b h) -> p b h",
                                             h=H)[0:rem, full:full + 1, :])

                npc = len(pieces)
                for pi in range(min(2, npc)):
                    gather(pi)
                for pi in range(npc):
                    if pi + 2 < npc:
                        gather(pi + 2)
                    stage(pi)

                nc.gpsimd.collective_compute(
                    "AllToAll", mybir.AluOpType.bypass, replica_groups=groups,
                    ins=[a2a_in[s].opt()],
                    outs=[table2[SEG_BASE[s]:SEG_BASE[s] + C * SEGS[s], :]])

            # ================= aggregation (shared by both layers) ===========
            def agg_group(g, lab, src_rows, src_self, stats, conv_after):
                """Aggregate group g: one-hot matmuls -> pg (feature-major),
                then hand the CH PSUM slabs to conv_after(g, gn, pgs)."""
                g0 = g * GRP
                gn = min(GRP, T_OWN - g0)
                nb = blk_off[g0 + gn] - blk_off[g0]
                ohg = wp.tile([P, (gn + nb) * P], BF16, tag="ohg", bufs=2,
                              name=f"ohg_{lab}_{g0}")
                nc.sync.dma_start(
                    out=ohg,
                    in_=oh_d[:, oh_goff[g]:oh_goff[g] + (gn + nb) * P])
                selfg = wp.tile([P, gn * H], BF16, tag="selfg", bufs=2,
                                name=f"selfg_{lab}_{g0}")
                nc.sync.dma_start(
                    out=selfg.rearrange("p (q h) -> p q h", h=H),
                    in_=src_self[g0 * P:(g0 + gn) * P, :].rearrange(
                        "(q p) h -> p q h", p=P))
                # edge rows: pre-gathered input (layer 1) or dma_gather
                # from the z2 table (layer 2), pieces of <= 8 blocks
                g4s = []
                for b0 in range(0, nb, 8):
                    bn_ = min(8, nb - b0)
                    g4 = wp.tile([P, bn_ * H], FP8, tag="g4", bufs=3,
                                 name=f"g4_{lab}_{g0}_{b0}")
                    if src_rows is None:
                        nc.sync.dma_start(
                            out=g4,
                            in_=xg_d[:, (blk_off[g0] + b0) * H:
                                     (blk_off[g0] + b0 + bn_) * H])
                    else:
                        ni = bn_ * P
                        i0 = (blk_off[g0] + b0) * P
                        nc.gpsimd.dma_gather(
                            out_ap=g4.rearrange("p (b h) -> p b h", h=H),
                            in_ap=src_rows,
                            idxs_ap=gidx_sb[:, i0 // 16:(i0 + ni) // 16],
                            num_idxs=ni, num_idxs_reg=ni, elem_size=H,
                            queue_num=next_q())
                    g4s.append(g4)
                outs = []
                for c in range(CH):
                    pg = pp.tile([P, GRP * P], F32, tag="agg", bufs=4,
                                 name=f"agg_{lab}_{g0}_{c}")
                    for q in range(gn):
                        t = g0 + q
                        nbt = blk_off[t + 1] - blk_off[t]
                        bb = blk_off[t] - blk_off[g0]
                        nc.tensor.matmul(
                            pg[:, q * P:(q + 1) * P],
                            lhsT=selfg[:, q * H + c * P:q * H + (c + 1) * P],
                            rhs=ohg[:, q * P:(q + 1) * P],
                            start=True, stop=(nbt == 0),
                            skip_group_check=True)
                        for b in range(nbt):
                            gb = bb + b
                            g4 = g4s[gb // 8]
                            lb = gb % 8
                            nc.tensor.matmul(
                                pg[:, q * P:(q + 1) * P],
                                lhsT=g4[:, lb * H + c * P:lb * H + (c + 1) * P],
                                rhs=ohg[:, (gn + gb) * P:(gn + gb + 1) * P],
                                start=False, stop=(b == nbt - 1),
                                skip_group_check=True)
                    if stats is not None:
                        nc.vector.bn_stats(
                            stats[:, (c * NG + g) * 6:(c * NG + g) * 6 + 6],
                            pg[:, 0:gn * P])
                    outs.append(pg)
                conv_after(g, gn, outs)

            # layer 1: agg over x then fused conv1 (feature-major)
            def conv1_after(g, gn, pgs):
                g0 = g * GRP
                xagg = wp.tile([P, CH * gn * P], BF16, tag="xagg", bufs=2,
                               name=f"xagg_{g0}")
                for c in range(CH):
                    nc.vector.tensor_copy(
                        xagg[:, c * gn * P:(c + 1) * gn * P],
                        pgs[c][:, 0:gn * P])
                for co in range(CH):
                    zt = pp.tile([P, GRP * P], F32, tag="zp", bufs=3,
                                 name=f"zt_{g0}_{co}")
                    for ci in range(CH):
                        last = (ci == CH - 1) and not d["has_bias"]
                        nc.tensor.matmul(
                            zt[:, 0:gn * P],
                            lhsT=wf_sb[:, ci * H + co * P:ci * H + (co + 1) * P],
                            rhs=xagg[:, ci * gn * P:(ci + 1) * gn * P],
                            start=(ci == 0), stop=last)
                    if d["has_bias"]:
                        nc.tensor.matmul(
                            zt[:, 0:gn * P],
                            lhsT=brow_sb[:, co * P:(co + 1) * P],
                            rhs=rowsum_sb[:, g0 * P:g0 * P + gn * P],
                            start=False, stop=True)
                    nc.vector.bn_stats(
                        stats_sb[0][:, (co * NG + g) * 6:(co * NG + g) * 6 + 6],
                        zt[:, 0:gn * P])
                    nc.scalar.activation(
                        aggT_sb[:, (g * CH + co) * GRP * P:
                                (g * CH + co) * GRP * P + gn * P],
                        zt[:, 0:gn * P], Copy)

            def agg2_after(g, gn, pgs):
                g0 = g * GRP
                for c in range(CH):
                    nc.scalar.activation(
                        aggT_sb[:, (g * CH + c) * GRP * P:
                                (g * CH + c) * GRP * P + gn * P],
                        pgs[c][:, 0:gn * P], Copy)

            # ================= BN coeffs =================
            def bn_coeffs(l):
                stats = stats_sb[l]
                mv = cp.tile([P, 2 * CH], F32, name=f"mv_{l}")
                for c in range(CH):
                    nc.vector.bn_aggr(
                        mv[:, 2 * c:2 * c + 2],
                        stats[:, c * NG * 6:(c + 1) * NG * 6])
                mu_l = mv.rearrange("p (c two) -> p c two", two=2)[:, :, 0]
                var_l = mv.rearrange("p (c two) -> p c two", two=2)[:, :, 1]
                sums = cp.tile([P, 2 * CH], F32, name=f"sums_{l}")
                m2 = cp.tile([P, CH], F32, name=f"m2_{l}")
                nc.vector.tensor_mul(m2, mu_l, mu_l)
                nc.vector.tensor_add(m2, var_l, m2)
                nc.vector.tensor_scalar_mul(sums[:, 0:CH], mu_l, float(OWN_PAD))
                nc.vector.tensor_scalar_mul(sums[:, CH:2 * CH], m2,
                                            float(OWN_PAD))
                nc.sync.dma_start(out=ar_in[l][:, :], in_=sums)
                nc.gpsimd.collective_compute(
                    "AllReduce", Add, replica_groups=groups,
                    ins=[ar_in[l].opt()], outs=[ar_out[l].opt()])
                sg = cp.tile([P, 2 * CH], F32, name=f"sg_{l}")
                nc.sync.dma_start(out=sg, in_=ar_out[l][:, :])
                mu = cp.tile([P, CH], F32, name=f"mu_{l}")
                nc.vector.tensor_scalar_mul(mu, sg[:, 0:CH], 1.0 / N)
                ex2 = cp.tile([P, CH], F32, name=f"ex2_{l}")
                nc.vector.tensor_scalar_mul(ex2, sg[:, CH:2 * CH], 1.0 / N)
                var = cp.tile([P, CH], F32, name=f"var_{l}")
                nc.vector.tensor_mul(var, mu, mu)
                nc.vector.tensor_sub(var, ex2, var)
                nc.vector.tensor_scalar_add(var, var, EPS)
                std = cp.tile([P, CH], F32, name=f"std_{l}")
                nc.scalar.activation(std, var, Sqrt)
                rstd = cp.tile([P, CH], F32, name=f"rstd_{l}")
                nc.vector.reciprocal(rstd, std)
                scale = cp.tile([P, CH], F32, name=f"scale_{l}")
                nc.vector.tensor_mul(scale, gmb_sb[:, 0:CH], rstd)
                shift = cp.tile([P, CH], F32, name=f"shift_{l}")
                nc.vector.tensor_mul(shift, mu, scale)
                nc.vector.tensor_sub(shift, gmb_sb[:, CH:2 * CH], shift)
                return scale, shift

            # ================= z2 producer =================
            def z2_phase(scale, shift):
                seg_ptr = [0]
                for g in range(NG):
                    g0 = g * GRP
                    gn = min(GRP, T_OWN - g0)
                    ht = wp.tile([P, CH * gn * P], BF16, tag="ht", bufs=2,
                                 name=f"ht_{g0}")
                    for c in range(CH):
                        nc.scalar.activation(
                            ht[:, c * gn * P:(c + 1) * gn * P],
                            aggT_sb[:, (g * CH + c) * GRP * P:
                                    (g * CH + c) * GRP * P + gn * P],
                            Lrelu, bias=shift[:, c:c + 1],
                            scale=scale[:, c:c + 1], alpha=ALPHA)
                    zh4 = wp.tile([P, gn * H], BF16, tag="zh4", bufs=2,
                                  name=f"zh4_{g0}")
                    for q in range(gn):
                        zp = pp.tile([P, H], F32, tag="zp", bufs=3,
                                     name=f"zp2_{g0}_{q}")
                        for c in range(CH):
                            nc.tensor.matmul(
                                zp,
                                lhsT=ht[:, (c * gn + q) * P:
                                        (c * gn + q + 1) * P],
                                rhs=wc_sb[:, c * H:(c + 1) * H],
                                start=(c == 0), stop=(c == CH - 1))
                        nc.vector.tensor_copy(zh4[:, q * H:(q + 1) * H], zp)
                    nc.sync.dma_start(
                        out=table2o[g0 * P:(g0 + gn) * P, :].rearrange(
                            "(q p) h -> p q h", p=P),
                        in_=zh4.rearrange("p (q h) -> p q h", h=H))
                    nc.gpsimd.dma_start(
                        out=table2[g0 * P:(g0 + gn) * P, :].rearrange(
                            "(q p) h -> p q h", p=P),
                        in_=zh4.rearrange("p (q h) -> p q h", h=H))
                    while (seg_ptr[0] < NSEG
                           and g + 1 == SEG_CFG[seg_ptr[0]]):
                        fill_and_a2a(seg_ptr[0])
                        seg_ptr[0] += 1

            # ================= head =================
            # Lrelu split across Scalar (chunks 0-2) and DVE (chunk 3); the
            # per-group [1, gn*P] head outputs bounce through DRAM and come
            # back transposed so ONE Sigmoid (one ACT table load) finishes
            # the job, baseline outcols-style.
            T_PAD = (T_OWN + 15) // 16 * 16
            pre_d = dp.tile([T_PAD, P], BF16, name="pre_d")
            Mul = mybir.AluOpType.mult
            AddOp = mybir.AluOpType.add
            Max = mybir.AluOpType.max

            def head(scale, shift):
                if T_PAD > T_OWN:
                    zpad = cp.tile([1, (T_PAD - T_OWN) * P], BF16,
                                   name="zpad")
                    nc.vector.memset(zpad[:, :], 0)
                    nc.sync.dma_start(
                        out=pre_d[T_OWN:T_PAD, :].rearrange(
                            "(a t) p -> a (t p)", a=1),
                        in_=zpad[:, :])
                for g in range(NG):
                    g0 = g * GRP
                    gn = min(GRP, T_OWN - g0)
                    ht = wp.tile([P, CH * gn * P], BF16, tag="ht", bufs=2,
                                 name=f"hto_{g0}")
                    for c in range(CH - 1):
                        nc.scalar.activation(
                            ht[:, c * gn * P:(c + 1) * gn * P],
                            aggT_sb[:, (g * CH + c) * GRP * P:
                                    (g * CH + c) * GRP * P + gn * P],
                            Lrelu, bias=shift[:, c:c + 1],
                            scale=scale[:, c:c + 1], alpha=ALPHA)
                    c = CH - 1
                    t1 = wp.tile([P, GRP * P], F32, tag="dvt", bufs=2,
                                 name=f"dvt_{g0}")
                    nc.vector.tensor_scalar(
                        t1[:, 0:gn * P],
                        aggT_sb[:, (g * CH + c) * GRP * P:
                                (g * CH + c) * GRP * P + gn * P],
                        scale[:, c:c + 1], shift[:, c:c + 1], Mul, AddOp)
                    nc.vector.scalar_tensor_tensor(
                        ht[:, c * gn * P:(c + 1) * gn * P],
                        t1[:, 0:gn * P], ALPHA, t1[:, 0:gn * P], Mul, Max)
                    op = pp.tile([1, GRP * P], F32, tag="zp", bufs=3,
                                 name=f"op_{g0}")
                    for c in range(CH):
                        nc.tensor.matmul(
                            op[:, 0:gn * P],
                            lhsT=wfo_sb[:, c:c + 1],
                            rhs=ht[:, c * gn * P:(c + 1) * gn * P],
                            start=(c == 0), stop=(c == CH - 1),
                            skip_group_check=True)
                    ob = wp.tile([1, GRP * P], BF16, tag="sig", bufs=2,
                                 name=f"ob_{g0}")
                    nc.vector.tensor_copy(ob[:, 0:gn * P], op[:, 0:gn * P])
                    nc.sync.dma_start(
                        out=pre_d[g0:g0 + gn, :].rearrange(
                            "(a t) p -> a (t p)", a=1),
                        in_=ob[:, 0:gn * P])
                preT = cp.tile([P, T_PAD], BF16, name="preT")
                nc.sync.dma_start(out=preT, in_=pre_d[:, :], transpose=True)
                outcols = cp.tile([P, T_PAD], F32, name="outcols")
                nc.scalar.activation(outcols[:, 0:T_OWN], preT[:, 0:T_OWN],
                                     Sigmoid, bias=float(d["bOf"]), scale=1.0)
                full_t = S // P
                rem = S - full_t * P
                if full_t:
                    nc.sync.dma_start(
                        out=out_ext[0:full_t * P, :].rearrange(
                            "(t p) one -> p (t one)", p=P),
                        in_=outcols[:, 0:full_t])
                if rem:
                    nc.sync.dma_start(
                        out=out_ext[full_t * P:S, :].rearrange(
                            "(q p) h -> p q h", p=rem),
                        in_=outcols[0:rem, full_t:full_t + 1].rearrange(
                            "p (q h) -> p q h", q=1))

            # ================= pipeline =================
            for g in range(NG):
                agg_group(g, "l1", None, x_own_d, None,
                          conv1_after)
            scale1, shift1 = bn_coeffs(0)
            z2_phase(scale1, shift1)
            for g in range(NG):
                agg_group(g, "l2", table2[0:TABLE_ROWS, :], table2o,
                          stats_sb[1], agg2_after)
            scale2, shift2 = bn_coeffs(1)
            head(scale2, shift2)

    nc.compile()
    return nc


# ---------------------------------------------------------------------------
# Entry point
# ---------------------------------------------------------------------------

_CACHE = {}


def _get_program(dims):
    key = tuple(sorted((k, str(v)) for k, v in dims.items()))
    if key not in _CACHE:
        _CACHE[key] = build_program(dims)
    return _CACHE[key]


def kernel(x, edge_index, W1, b1, Wc, bc, gamma, beta, W2, b2, WO, bO,
           trace=False):
    dims, in_maps = make_plan(x, edge_index, W1, b1, Wc, bc, gamma, beta,
                              W2, b2, WO, bO)
    nc = _get_program(dims)
    res = run_bass_kernel_spmd(nc, in_maps, core_ids=list(range(dims["C"])),
                               trace=trace)
    out = np.concatenate([r["out"] for r in res.results], axis=0)
    kernel.last_results = res
    return out.astype(np.float32)
